# revision 4
# baseline (speedup 1.0000x reference)
"""GAT layer (4 heads x 32 dims, concat) on 8 trn2 NeuronCores.

Strategy (edge/data parallel, dst-sharded):
  - Nodes padded to 100352 = 8 cores x 98 blocks x 128; core c owns dst
    range [c*12544, (c+1)*12544).
  - Phase A (sharded): each core computes h_ext = X_shard @ [W | W*a_src
    | W*a_dst] in fp32 on the PE, emits a 512B/row gather table
    [h bf16(256B) | a_src f32x4 | a_dst f32x4 | pad], then an AllGather
    replicates the full 100352-row table to every core.  a_dst
    additionally goes to a core-local [12544, 64] f32 table.
  - Phase B: edges are host-bucketed by (dst block, src quarter) into
    fixed-capacity buckets.  Per (superblock=7 blocks, quarter) one
    dma_gather pulls h+a_src rows by src (int16 quarter-local indices)
    and a second pulls a_dst rows by dst (core-local indices).  Scores =
    a_src + a_dst -> LeakyReLU(0.2) -> exp (f32 exact), messages =
    h_bf16 * p, and a per-tile one-hot selection matrix S_T (built with
    one is_equal against an iota row) turns the per-dst-block segment
    sum into PE matmuls accumulating [sum p*h | sum p] in PSUM.
  - Block end: out = num / (den + 1e-16), DMA'd to the core's output
    shard; host concatenates and trims to 100000 rows.
"""

import numpy as np
import ml_dtypes

import concourse.bass as bass
import concourse.bacc as bacc
import concourse.mybir as mybir
import concourse.tile as tile
from concourse.bass_utils import run_bass_kernel_spmd
from contextlib import ExitStack

P = 128
N_NODES = 100000
N_PAD = 100352            # 8 * 98 * 128
E_EDGES = 1600000
D_IN = 128
N_HEADS = 4
HEAD_DIM = 32
D_OUT = 128
NEG_SLOPE = 0.2
EPS = 1e-16
NCORES = 8
NODES_PER_CORE = N_PAD // NCORES          # 12544
BLOCKS_PER_CORE = NODES_PER_CORE // P     # 98
SB_BLOCKS = 7                             # blocks per superblock
N_SB = BLOCKS_PER_CORE // SB_BLOCKS       # 14
QUARTER = N_PAD // 4                      # 25088
ROW_G1 = 256                              # bf16 elems (512B) per table row
ROW_G2 = 64                               # f32 elems (256B) per a_dst row
BF16 = ml_dtypes.bfloat16

SINGLE_PACKET = False


def _build_program(b_cap: int):
    """One SPMD program; all shape-determining numbers are compile-time."""
    nt_bucket = b_cap // P                 # tiles per (block, quarter) bucket
    n_idx = SB_BLOCKS * b_cap              # indices per gather call
    nt_call = n_idx // P                   # tiles per call
    n_calls = N_SB * 4
    w16 = n_idx // 16

    nc = bacc.Bacc("TRN2", target_bir_lowering=False, debug=False,
                   num_devices=NCORES)
    xts = nc.declare_dram_parameter("xts", [P, NODES_PER_CORE], mybir.dt.float32, isOutput=False)
    wcat = nc.declare_dram_parameter("wcat", [P, 136], mybir.dt.float32, isOutput=False)
    g1idx = nc.declare_dram_parameter("g1idx", [n_calls, P, w16], mybir.dt.int16, isOutput=False)
    g2idx = nc.declare_dram_parameter("g2idx", [n_calls, P, w16], mybir.dt.int16, isOutput=False)
    dstloc = nc.declare_dram_parameter("dstloc", [n_calls, P, nt_call], mybir.dt.bfloat16, isOutput=False)
    out_ext = nc.declare_dram_parameter("out", [NODES_PER_CORE, D_OUT], mybir.dt.float32, isOutput=True)

    tshard = nc.dram_tensor("tshard", [NODES_PER_CORE, ROW_G1], mybir.dt.bfloat16)
    tableg = nc.dram_tensor("tableg", [N_PAD, ROW_G1], mybir.dt.bfloat16)
    atable = nc.dram_tensor("atable", [NODES_PER_CORE, ROW_G2], mybir.dt.float32)

    with tile.TileContext(nc) as tc, ExitStack() as ctx:
        const_p = ctx.enter_context(tc.tile_pool(name="const", bufs=1))
        sb = ctx.enter_context(tc.tile_pool(name="sbp", bufs=2))

        # constants
        wc = const_p.tile([P, 136], mybir.dt.float32)
        nc.sync.dma_start(out=wc[:], in_=wcat[:])
        iota_i = const_p.tile([P, P], mybir.dt.int32)
        nc.gpsimd.iota(iota_i[:], pattern=[[1, P]], base=0, channel_multiplier=0)
        iota_bf = const_p.tile([P, P], mybir.dt.bfloat16)
        nc.vector.tensor_copy(out=iota_bf[:], in_=iota_i[:])

        # ---------------- Phase A ----------------
        ctx_a = ExitStack()
        pa = ctx_a.enter_context(tc.tile_pool(name="pa", bufs=3))
        pap = ctx_a.enter_context(tc.tile_pool(name="pap", bufs=2, space="PSUM"))
        for k in range(BLOCKS_PER_CORE):
            xc = pa.tile([P, P], mybir.dt.float32, tag="xc")
            nc.sync.dma_start(out=xc[:], in_=xts[:, k * P:(k + 1) * P])
            hp = pap.tile([P, 136], mybir.dt.float32, tag="hp")
            nc.tensor.matmul(out=hp[:], lhsT=xc[:], rhs=wc[:], start=True, stop=True)
            rowt = pa.tile([P, ROW_G1], mybir.dt.bfloat16, tag="rowt")
            nc.vector.tensor_copy(out=rowt[:, 0:128], in_=hp[:, 0:128])
            asc = pa.tile([P, 8], mybir.dt.float32, tag="asc")
            nc.vector.tensor_copy(out=asc[:], in_=hp[:, 128:136])
            nc.vector.tensor_copy(out=rowt[:, 128:144], in_=asc[:].bitcast(mybir.dt.bfloat16))
            # zero the pad so gathered garbage can never be NaN
            nc.vector.memset(rowt[:, 144:ROW_G1], 0)
            nc.sync.dma_start(out=tshard[k * P:(k + 1) * P, :], in_=rowt[:])
            adt = pa.tile([P, 4], mybir.dt.float32, tag="adt")
            nc.vector.tensor_copy(out=adt[:], in_=hp[:, 132:136])
            nc.sync.dma_start(
                out=bass.AP(atable[:].tensor, k * P * ROW_G2, [[ROW_G2, P], [1, 4]]),
                in_=adt[:])

        nc.gpsimd.collective_compute(
            "AllGather", mybir.AluOpType.bypass,
            replica_groups=[list(range(NCORES))],
            ins=[tshard[:]], outs=[tableg[:]],
        )
        ctx_a.close()
        psb = ctx.enter_context(tc.tile_pool(name="psb", bufs=1, space="PSUM"))

        # ---------------- Phase B ----------------
        for s in range(N_SB):
            psums = [psb.tile([P, 132], mybir.dt.float32, tag=f"blk{j}", name=f"ps_{s}_{j}")
                     for j in range(SB_BLOCKS)]
            for q in range(4):
                call = s * 4 + q
                i1 = sb.tile([P, w16], mybir.dt.int16, tag="i1")
                nc.sync.dma_start(out=i1[:], in_=g1idx[call])
                g1 = sb.tile([P, nt_call * ROW_G1], mybir.dt.bfloat16, tag="g1")
                nc.gpsimd.dma_gather(
                    out_ap=g1[:].rearrange("p (k r) -> p k r", r=ROW_G1),
                    in_ap=tableg[q * QUARTER:(q + 1) * QUARTER, :],
                    idxs_ap=i1[:], num_idxs=n_idx, num_idxs_reg=n_idx,
                    elem_size=ROW_G1, single_packet=SINGLE_PACKET)
                i2 = sb.tile([P, w16], mybir.dt.int16, tag="i2")
                nc.sync.dma_start(out=i2[:], in_=g2idx[call])
                g2 = sb.tile([P, nt_call * ROW_G2], mybir.dt.float32, tag="g2")
                nc.gpsimd.dma_gather(
                    out_ap=g2[:].rearrange("p (k r) -> p k r", r=ROW_G2),
                    in_ap=atable[:], idxs_ap=i2[:], num_idxs=n_idx,
                    num_idxs_reg=n_idx, elem_size=ROW_G2,
                    single_packet=SINGLE_PACKET)
                dl = sb.tile([P, nt_call], mybir.dt.bfloat16, tag="dl")
                nc.sync.dma_start(out=dl[:], in_=dstloc[call])

                g1v = g1[:].rearrange("p (k r) -> p k r", r=ROW_G1)
                g2v = g2[:].rearrange("p (k r) -> p k r", r=ROW_G2)
                # scores (f32 exact)
                sc = sb.tile([P, nt_call * 4], mybir.dt.float32, tag="sc")
                nc.vector.tensor_tensor(
                    out=sc[:].rearrange("p (k h) -> p k h", h=4),
                    in0=g1v[:, :, 128:136].bitcast(mybir.dt.float32),
                    in1=g2v[:, :, 0:4], op=mybir.AluOpType.add)
                t1 = sb.tile([P, nt_call * 4], mybir.dt.float32, tag="t1")
                nc.vector.tensor_scalar(out=t1[:], in0=sc[:], scalar1=0.0,
                                        scalar2=None, op0=mybir.AluOpType.max)
                t2 = sb.tile([P, nt_call * 4], mybir.dt.float32, tag="t2")
                nc.vector.tensor_scalar(out=t2[:], in0=sc[:], scalar1=NEG_SLOPE,
                                        scalar2=0.0, op0=mybir.AluOpType.mult,
                                        op1=mybir.AluOpType.min)
                lr = sb.tile([P, nt_call * 4], mybir.dt.float32, tag="lr")
                nc.vector.tensor_tensor(out=lr[:], in0=t1[:], in1=t2[:],
                                        op=mybir.AluOpType.add)
                pb = sb.tile([P, nt_call * 4], mybir.dt.bfloat16, tag="pb")
                nc.scalar.activation(out=pb[:], in_=lr[:],
                                     func=mybir.ActivationFunctionType.Exp)
                # selection matrix
                st = sb.tile([P, nt_call * P], mybir.dt.bfloat16, tag="st")
                nc.vector.tensor_tensor(
                    out=st[:].rearrange("p (k n) -> p k n", n=P),
                    in0=dl[:].unsqueeze(-1).to_broadcast([P, nt_call, P]),
                    in1=iota_bf[:].unsqueeze(1).to_broadcast([P, nt_call, P]),
                    op=mybir.AluOpType.is_equal)
                # rhs = [msg | p]
                rhs = sb.tile([P, nt_call * 132], mybir.dt.bfloat16, tag="rhs")
                rhsv = rhs[:].rearrange("p (k r) -> p k r", r=132)
                pbv = pb[:].rearrange("p (k h) -> p k h", h=4)
                for h in range(N_HEADS):
                    p_rep = bass.AP(pb[:].tensor, pb[:].offset + h,
                                    [pb[:].ap[0], [4, nt_call], [0, 32]])
                    nc.vector.tensor_tensor(
                        out=rhsv[:, :, h * 32:(h + 1) * 32],
                        in0=g1v[:, :, h * 32:(h + 1) * 32],
                        in1=p_rep,
                        op=mybir.AluOpType.mult)
                nc.vector.tensor_copy(out=rhsv[:, :, 128:132], in_=pbv)
                # scatter matmuls
                for t in range(nt_call):
                    j = t // nt_bucket
                    nc.tensor.matmul(
                        out=psums[j][:],
                        lhsT=st[:, t * P:(t + 1) * P],
                        rhs=rhs[:, t * 132:(t + 1) * 132],
                        start=(q == 0 and t % nt_bucket == 0),
                        stop=(q == 3 and t % nt_bucket == nt_bucket - 1),
                    )
            # block-end normalize
            for j in range(SB_BLOCKS):
                den = sb.tile([P, 4], mybir.dt.float32, tag="den")
                nc.vector.tensor_scalar(out=den[:], in0=psums[j][:, 128:132],
                                        scalar1=EPS, scalar2=None,
                                        op0=mybir.AluOpType.add)
                rec = sb.tile([P, 4], mybir.dt.float32, tag="rec")
                nc.vector.reciprocal(out=rec[:], in_=den[:])
                ob = sb.tile([P, D_OUT], mybir.dt.float32, tag="ob")
                for h in range(N_HEADS):
                    nc.vector.tensor_tensor(
                        out=ob[:, h * 32:(h + 1) * 32],
                        in0=psums[j][:, h * 32:(h + 1) * 32],
                        in1=rec[:, h:h + 1].to_broadcast([P, 32]),
                        op=mybir.AluOpType.mult)
                blk = s * SB_BLOCKS + j
                nc.sync.dma_start(out=out_ext[blk * P:(blk + 1) * P, :], in_=ob[:])

    nc.compile()
    return nc


def _wrap_idx(arr, n_idx):
    """[..., n_idx] int16 -> [..., 128, n_idx//16] Q7 wrap layout."""
    lead = arr.shape[:-1]
    w = n_idx // 16
    a = arr.reshape(*lead, w, 16)
    a = np.swapaxes(a, -1, -2)                      # [..., 16, w]
    return np.tile(a, (1,) * len(lead) + (8, 1)).reshape(*lead, 128, w)


def kernel(node_features, edge_index, W, a):
    node_features = np.asarray(node_features, dtype=np.float32)
    edge_index = np.asarray(edge_index)
    W = np.asarray(W, dtype=np.float32)
    a = np.asarray(a, dtype=np.float32)

    # ---- host param folding
    w_asrc = np.stack([W[:, h * HEAD_DIM:(h + 1) * HEAD_DIM] @ a[h, :HEAD_DIM]
                       for h in range(N_HEADS)], axis=1)          # [128, 4]
    w_adst = np.stack([W[:, h * HEAD_DIM:(h + 1) * HEAD_DIM] @ a[h, HEAD_DIM:]
                       for h in range(N_HEADS)], axis=1)          # [128, 4]
    wcat = np.concatenate([W, w_asrc, w_adst], axis=1).astype(np.float32)  # [128,136]

    xt = np.zeros((D_IN, N_PAD), dtype=np.float32)
    xt[:, :N_NODES] = node_features.T

    # ---- edge bucketing
    src = edge_index[0].astype(np.int64)
    dst = edge_index[1].astype(np.int64)
    blk = dst // P                       # global block 0..783
    q = src // QUARTER                   # quarter 0..3
    bid = blk * 4 + q                    # bucket 0..3135
    nbuckets = (N_PAD // P) * 4
    counts = np.bincount(bid, minlength=nbuckets)
    b_cap = int(np.ceil(counts.max() / P) * P)
    nt_bucket = b_cap // P
    n_idx = SB_BLOCKS * b_cap
    nt_call = n_idx // P
    n_calls = N_SB * 4

    order = np.argsort(bid, kind="stable")
    starts = np.zeros(nbuckets, dtype=np.int64)
    starts[1:] = np.cumsum(counts)[:-1]
    pos_in = np.arange(E_EDGES, dtype=np.int64) - np.repeat(starts, counts)
    slot = np.empty(E_EDGES, dtype=np.int64)
    slot[order] = bid[order] * b_cap + pos_in

    total_slots = nbuckets * b_cap
    s_src16 = np.zeros(total_slots, dtype=np.int16)
    s_dstl16 = np.zeros(total_slots, dtype=np.int16)
    s_dstb = np.full(total_slots, -1.0, dtype=np.float32)
    s_src16[slot] = (src - q * QUARTER).astype(np.int16)
    s_dstl16[slot] = (dst % NODES_PER_CORE).astype(np.int16)
    s_dstb[slot] = (dst % P).astype(np.float32)

    # reshape to per-core call layout: core -> [392 buckets, b_cap]
    # call (s, q) covers buckets (blk=s*7+j, q) j=0..6 in j-major order
    def core_calls(arr):
        # arr [total_slots] -> [NCORES, n_calls, n_idx]
        a4 = arr.reshape(NCORES, BLOCKS_PER_CORE, 4, b_cap)       # [c, blk, q, cap]
        a5 = a4.reshape(NCORES, N_SB, SB_BLOCKS, 4, b_cap)
        a6 = np.swapaxes(a5, 2, 3)                                # [c, sb, q, j, cap]
        return a6.reshape(NCORES, n_calls, n_idx)

    g1_flat = core_calls(s_src16)
    g2_flat = core_calls(s_dstl16)
    dl_flat = core_calls(s_dstb)

    g1idx = _wrap_idx(g1_flat.reshape(-1, n_idx), n_idx).reshape(NCORES, n_calls, P, n_idx // 16)
    g2idx = _wrap_idx(g2_flat.reshape(-1, n_idx), n_idx).reshape(NCORES, n_calls, P, n_idx // 16)
    # dstloc partition-major: list position i = t*128 + p -> [p, t]
    dlp = dl_flat.reshape(NCORES, n_calls, nt_call, P)
    dlp = np.swapaxes(dlp, 2, 3).astype(BF16)                     # [c, call, P, nt]

    nc = _build_program(b_cap)
    in_maps = []
    for c in range(NCORES):
        in_maps.append(dict(
            xts=np.ascontiguousarray(xt[:, c * NODES_PER_CORE:(c + 1) * NODES_PER_CORE]),
            wcat=wcat,
            g1idx=np.ascontiguousarray(g1idx[c]),
            g2idx=np.ascontiguousarray(g2idx[c]),
            dstloc=np.ascontiguousarray(dlp[c]),
        ))
    res = run_bass_kernel_spmd(nc, in_maps, core_ids=list(range(NCORES)))
    out = np.concatenate([res.results[c]["out"] for c in range(NCORES)], axis=0)
    return np.ascontiguousarray(out[:N_NODES]).astype(np.float32)
